# revision 1
# baseline (speedup 1.0000x reference)
"""AttentionAugmentation2D kernel for 8 Trainium2 NeuronCores — v3.

Data-parallel over batch (B=8 -> 1 batch element per core).

Math (per batch, per head; H=W=32, L=H*W=1024, dh=32):
  logits[(x,y),(x',y')] = q.k + q.krw[y'-y+31] + q.krh[x'-x+31]
Both relative terms are folded into a single K=96 matmul:
  Q_aug = [qT; skew_w(q @ krw^T); skew_h(q @ krh^T)]   (96 x 1024 per head)
  K_aug = [kT; onehot32(y'); onehot32(x')]             (96 x 1024 per head)
logits are computed transposed (keys on partitions) so that exp(logitsT)
is directly the rhs of the attention@V matmul.

v3 structure:
  - the whole input ships as one bf16 [L, 768] host tensor; q/k are
    transposed by the DMA XBAR directly DRAM -> SBUF (zero engine/PE
    time) and V is DMA'd straight into its [128, t, h, d] layout.
    The XBAR stream writes [d, pos*4 + headblock]; the aug tensors use
    that interleaved column layout and all matmuls read stride-4
    slices. After the QK matmul the interleave vanishes (output order
    follows the rhs stream), so exp/attention@V/finish see pos order.
  - bf16 matmul inputs run at 1 cycle/row at ANY free size, so the rel
    matmuls are split per 4-head half: half 0 runs as soon as the first
    q transpose lands, half 1 is interleaved into attention phase 0
    (its PSUM tiles ride the lt pool ring); heads 4-7 only need it
    ~4 phases later.
  - exp runs on Pool (qc0) and ACT (qc1) via raw InstActivation with
    the 1/sqrt(dh) scale folded into the activation scale operand.
  - attention@V accumulates into per-head [33,L] PSUM tiles from a
    2-deep pool; per-head finish = 2 evac halves (DVE), 8 PE transposes
    into one PSUM tile, strided reciprocal, broadcast multiply.
"""

import math
import numpy as np
import ml_dtypes

import concourse.bass as bass
import concourse.mybir as mybir
import concourse.tile as tile
from concourse import bacc
from concourse.bass_utils import run_bass_kernel_spmd

FP = mybir.dt.float32
FPR = mybir.dt.float32r
BF = mybir.dt.bfloat16
AF = mybir.ActivationFunctionType

B = 8
H = W = 32
NH = 8
DH = 32          # per-head depth for q/k/v
L = H * W        # 1024 positions
SCALE = float(DH) ** -0.5
NT = L // 128    # 8 position tiles


def _build_onehot():
    # rows 0-31: onehot of y' = key % 32 ; rows 32-63: onehot of x' = key//32
    # pre-interleaved to the XBAR column layout col = pos*4 + hb, bf16 exact
    oh = np.zeros((64, L), dtype=np.float32)
    k = np.arange(L)
    oh[k % 32, k] = 1.0
    oh[32 + k // 32, k] = 1.0
    ohi = np.repeat(oh[:, :, None], 4, axis=2).reshape(64, 4 * L)
    return np.ascontiguousarray(ohi.astype(ml_dtypes.bfloat16))


def _build_nc():
    nc = bacc.Bacc(
        "TRN2",
        target_bir_lowering=False,
        debug=False,
        enable_asserts=True,
        num_devices=B,
    )
    xbf = nc.declare_dram_parameter("xbf", [L, 3 * NH * DH], BF, isOutput=False)
    qki = nc.declare_dram_parameter("qki", [128, 4 * L], BF, isOutput=False)
    krwh = nc.declare_dram_parameter("krwhT", [DH, 2 * (2 * W - 1)], BF, isOutput=False)
    oneh = nc.declare_dram_parameter("oneh", [64, 4 * L], BF, isOutput=False)
    identb = nc.declare_dram_parameter("identb", [128, 128], BF, isOutput=False)
    out = nc.declare_dram_parameter("out", [L, NH * DH], FP, isOutput=True)

    def mkexp(eng, out_ap, in_ap, tag):
        # raw InstActivation so exp can run on Pool too; scale folds the
        # 1/sqrt(dh) so all upstream copies stay pure
        eng.add_instruction(
            mybir.InstActivation(
                name=f"vexp_{tag}",
                func=AF.Exp,
                ins=[
                    eng.lower_ap(in_ap),
                    mybir.ImmediateValue(dtype=FP, value=0.0),
                    mybir.ImmediateValue(dtype=FP, value=SCALE),
                    mybir.ImmediateValue(dtype=FP, value=0.0),
                ],
                outs=[eng.lower_ap(out_ap)],
            )
        )

    def copy_on(eng, dst, src):
        if eng is nc.scalar:
            eng.copy(dst, src)
        else:
            eng.tensor_copy(dst, src)

    with tile.TileContext(nc) as tc:
        with (
            tc.tile_pool(name="const", bufs=1) as cp,
        ):
            ident = cp.tile([128, 128], BF)
            krwh_sb = cp.tile([DH, 2 * (2 * W - 1)], BF)
            krw_sb = krwh_sb[:, 0 : 2 * W - 1]
            krh_sb = krwh_sb[:, 2 * W - 1 :]

            # interleaved column layout: col(half, pos, hb) =
            #   half*4096 + pos*4 + hb,  head h = half*4 + hb
            QaugT = cp.tile([96, 2, L, 4], BF)
            KaugT = cp.tile([96, 2, L, 4], BF)
            Vaug = cp.tile([128, NT, NH, DH + 2], BF)

            # The DMA transfers serialize on one pipe, and the scheduler
            # round-robins SWDGE/HWDGE with ~2us link latency on each
            # switch — so keep the whole chain on HWDGE, in deadline
            # order: half-0 data first, then V, half-1, and the SWDGE
            # ident (needed ~20us in) dead last.
            # q/k arrive host-pre-transposed+interleaved, packed as four
            # 32-row groups across 128 partitions: one fast DMA, then four
            # partition-shift copies (DVE 4x for half-0, Pool for half-1)
            qkst = cp.tile([128, 4 * L], BF, name="qkst")
            with tc.high_priority():
                nc.sync.dma_start(out=qkst, in_=qki[:])
                nc.sync.dma_start(out=krwh_sb, in_=krwh[:])
                nc.sync.dma_start(
                    out=KaugT[32:96, 0].rearrange("p f h -> p (f h)"), in_=oneh[:]
                )
                nc.sync.dma_start(out=ident, in_=identb[:])
            # V straight into its SBUF layout (leaves the ones column gap);
            # per-t pieces keep the DMA APs within 3 dims
            xvr = xbf.rearrange("(t p) c -> p t c", p=128)
            with tc.tile_wait_until(0.006):
                for t in range(NT):
                    nc.sync.dma_start(
                        out=Vaug[:, t, :, 0:DH],
                        in_=xvr[:, t, 512:768].rearrange("p (h d) -> p h d", d=DH),
                    )
            with tc.tile_wait_until(0.010):
                nc.sync.dma_start(
                    out=KaugT[32:96, 1].rearrange("p f h -> p (f h)"), in_=oneh[:]
                )
            # ones column for the softmax denominator: engine memset, no DMA
            nc.vector.memset(
                Vaug[:, :, :, DH : DH + 1].rearrange("p t h o -> p (t h o)"), 1.0
            )

            # rows: 0:32 q half0, 32:64 q half1, 64:96 k half0, 96:128 k half1
            nc.vector.tensor_copy(
                QaugT[0:32, 0].rearrange("p f h -> p (f h)"), qkst[0:32, :]
            )
            nc.vector.tensor_copy(
                KaugT[0:32, 0].rearrange("p f h -> p (f h)"), qkst[64:96, :]
            )
            nc.gpsimd.tensor_copy(
                QaugT[0:32, 1].rearrange("p f h -> p (f h)"), qkst[32:64, :]
            )
            nc.gpsimd.tensor_copy(
                KaugT[0:32, 1].rearrange("p f h -> p (f h)"), qkst[96:128, :]
            )

            out_sb = cp.tile([128, NT, NH * DH], FP)
            # (e^SCALE)^logit == exp(SCALE*logit): lets the DVE compute the
            # softmax exp as a TensorTensor pow with a broadcast const base
            ebase = cp.tile([128, 1], FP)
            nc.vector.memset(ebase, math.exp(SCALE))

            # rel views (interleaved): free ordering per mm is (hb, x|y)
            q_i = QaugT[0:32]                                  # [32,2,L,4]
            qr = q_i.rearrange("p a (x y) h -> p a h x y", y=W)
            wd = QaugT[32:64].rearrange("p a (x y) h -> p a h x y", y=W)
            hd = QaugT[64:96].rearrange("p a (x y) h -> p a h x y", y=W)

            def rel_group(pool, half, g, wdir, evac_engs, tag="rp"):
                # one y(or x)-group of 4 pre-skewed rel matmuls for one
                # 4-head half, then 2 evac halves
                rp = pool.tile([32, 4, 4, 32], FP, tag=tag, name=f"rp{half}_{wdir}_{g}")
                for i in range(4):
                    v = 4 * g + i
                    if wdir:
                        nc.tensor.matmul(
                            rp[:, i],
                            lhsT=krw_sb[:, 31 - v : 63 - v],
                            rhs=qr[:, half, :, :, v],
                            start=True,
                            stop=True,
                        )
                    else:
                        nc.tensor.matmul(
                            rp[:, i],
                            lhsT=krh_sb[:, 31 - v : 63 - v],
                            rhs=qr[:, half, :, v, :],
                            start=True,
                            stop=True,
                        )
                if wdir:
                    dst = wd[:, half, :, :, 4 * g : 4 * g + 4].rearrange(
                        "p h x i -> p i h x"
                    )
                else:
                    dst = hd[:, half, :, 4 * g : 4 * g + 4, :].rearrange(
                        "p h i y -> p i h y"
                    )
                copy_on(evac_engs[0], dst, rp)

            # ---------------- rel half 0 (heads 0-3) ----------------------
            rel0_rot = [(nc.vector, nc.scalar), (nc.scalar, nc.vector),
                        (nc.vector, nc.scalar), (nc.scalar, nc.vector)]
            with tc.tile_pool(name="ps_rel", bufs=6, space="PSUM") as ps_rel:
                for g in range(8):
                    rel_group(ps_rel, 0, g, True, rel0_rot[g % 4])
                for g in range(8):
                    rel_group(ps_rel, 0, g, False, rel0_rot[(g + 1) % 4])

            # ---------------- attention over heads ------------------------
            with (
                tc.tile_pool(name="wt", bufs=3) as wtp,
                tc.tile_pool(name="at", bufs=2) as atp,
                tc.tile_pool(name="sm", bufs=4) as smp,
                tc.tile_pool(name="stg", bufs=2) as stp,
                tc.tile_pool(name="ps_lt", bufs=2, space="PSUM") as ps_lt,
                tc.tile_pool(name="ps_av", bufs=2, space="PSUM") as ps_av,
            ):
                wts = {}
                avs = {}
                at_sbs = {}

                def evac_head(h, engs):
                    av = avs.pop(h)
                    at_sb = atp.tile([DH + 1, L], BF, tag="at", name=f"at{h}")
                    at_sbs[h] = at_sb
                    copy_on(engs[0], at_sb[:, 0:512], av[:, 0:512])
                    copy_on(engs[1], at_sb[:, 512:1024], av[:, 512:1024])

                def finish_ft(h, t0, t1, tt_eng):
                    at_sb = at_sbs[h]
                    ftile = ps_lt.tile(
                        [128, t1 - t0, DH + 2], BF, tag="lt", name=f"ft{h}_{t0}"
                    )
                    for t in range(t0, t1):
                        nc.tensor.transpose(
                            ftile[:, t - t0, 0 : DH + 1],
                            at_sb[:, t * 128 : (t + 1) * 128],
                            ident[0 : DH + 1, 0 : DH + 1],
                        )
                    rcp = smp.tile([128, t1 - t0], FP, tag="rcp")
                    nc.vector.reciprocal(rcp, ftile[:, :, DH])
                    rcp_b = bass.AP(
                        tensor=rcp.tensor,
                        offset=rcp.offset,
                        ap=[rcp.ap[0], rcp.ap[1], [0, DH]],
                    )
                    tt_eng.tensor_tensor(
                        out_sb[:, t0:t1, h * DH : (h + 1) * DH],
                        ftile[:, :, 0:DH],
                        rcp_b,
                        mybir.AluOpType.mult,
                    )

                # rel half 1 groups spread over phases 0-3 (heads 4-7 only
                # need them from phase 4); evacs on DVE (Pool cannot access
                # PSUM on real hardware)
                rel1 = [(g, True) for g in range(8)] + [(g, False) for g in range(8)]
                rel1_rot = [(nc.vector, nc.vector), (nc.vector, nc.vector)]

                for h in range(NH + 1):
                    if h < NH:
                        wts[h] = wtp.tile(
                            [128, NT, L], BF, tag="wt", name=f"wt{h}"
                        )
                        ha, hb = h // 4, h % 4
                    if h > 0:
                        avp = ps_av.tile([DH + 1, L], FP, tag="av", name=f"av{h-1}")
                        avs[h - 1] = avp
                        WTp = wts[h - 1]
                    for kt in range(NT):
                        if kt == 2 and h >= 2:
                            finish_ft(h - 2, 0, NT, nc.vector)
                            del at_sbs[h - 2]
                        if h < NH:
                            lt = ps_lt.tile([128, L], FP, tag="lt")
                            for qc in range(2):
                                nc.tensor.matmul(
                                    lt[:, qc * 512 : (qc + 1) * 512],
                                    lhsT=KaugT[:, ha, kt * 128 : (kt + 1) * 128, hb],
                                    rhs=QaugT[:, ha, qc * 512 : (qc + 1) * 512, hb],
                                    start=True,
                                    stop=True,
                                )
                            # softmax exp: ACT native Exp, plus a GPSIMD
                            # pow path ((e^SCALE)^x, ISA-legal on Pool) fed
                            # by a DVE PSUM->SBUF stage for 3 of 8 kts
                            if kt in (1, 4, 6):
                                stg = stp.tile([128, L], FP, tag="stg")
                                nc.vector.tensor_copy(stg, lt)
                                eb = bass.AP(
                                    tensor=ebase.tensor,
                                    offset=ebase.offset,
                                    ap=[ebase.ap[0], [0, L]],
                                )
                                nc.gpsimd.tensor_tensor(
                                    wts[h][:, kt, :], eb, stg,
                                    mybir.AluOpType.pow,
                                )
                            else:
                                nc.scalar.activation(
                                    wts[h][:, kt, :], lt, AF.Exp, scale=SCALE
                                )
                        if h < 4 and kt % 2 == 0:
                            g, wdir = rel1[4 * h + kt // 2]
                            rel_group(ps_av, 1, g, wdir, rel1_rot[kt % 2], tag="av")
                        if h > 0:
                            for qc in range(2):
                                nc.tensor.matmul(
                                    avp[:, qc * 512 : (qc + 1) * 512],
                                    lhsT=Vaug[:, kt, h - 1, 0 : DH + 1],
                                    rhs=WTp[:, kt, qc * 512 : (qc + 1) * 512],
                                    start=(kt == 0),
                                    stop=(kt == NT - 1),
                                )
                    if h > 0:
                        del wts[h - 1]
                        evac_head(h - 1, (nc.vector, nc.scalar))

                # tail: last head's finish interleaved with the out stores
                out_r = out.rearrange("(t p) c -> p t c", p=128)
                finish_ft(NH - 1, 0, NT // 2, nc.vector)
                for t in range(0, NT // 2, 2):
                    eng = (nc.sync, nc.scalar)[(t // 2) % 2]
                    eng.dma_start(
                        out=out_r[:, t : t + 2, :], in_=out_sb[:, t : t + 2, :]
                    )
                finish_ft(NH - 1, NT // 2, NT, nc.vector)
                for t in range(NT // 2, NT, 2):
                    eng = (nc.sync, nc.scalar)[(t // 2) % 2]
                    eng.dma_start(
                        out=out_r[:, t : t + 2, :], in_=out_sb[:, t : t + 2, :]
                    )
    nc.compile()
    return nc


_NC_CACHE = None


def kernel(inputs: np.ndarray, key_rel_w: np.ndarray, key_rel_h: np.ndarray) -> np.ndarray:
    global _NC_CACHE
    xf32 = inputs.astype(np.float32).reshape(B, L, 3 * NH * DH)
    xbf = np.ascontiguousarray(xf32.astype(ml_dtypes.bfloat16))
    # [g, hb, d, pos] -> [g*32+d, pos*4+hb]
    qki = np.ascontiguousarray(
        xf32[:, :, 0:512].transpose(0, 2, 1).reshape(B, 4, 4, 32, L)
        .transpose(0, 1, 3, 4, 2).reshape(B, 128, 4 * L)
        .astype(ml_dtypes.bfloat16)
    )
    krwhT = np.ascontiguousarray(
        np.concatenate([key_rel_w, key_rel_h], axis=0)
        .astype(np.float32).T.astype(ml_dtypes.bfloat16)
    )
    oneh = _build_onehot()

    if _NC_CACHE is None:
        _NC_CACHE = _build_nc()
    nc = _NC_CACHE

    identb = np.eye(128, dtype=np.float32).astype(ml_dtypes.bfloat16)
    in_maps = [
        {"xbf": xbf[b], "qki": qki[b], "krwhT": krwhT, "oneh": oneh,
         "identb": identb}
        for b in range(B)
    ]
    res = run_bass_kernel_spmd(nc, in_maps, list(range(B)))
    o = np.stack([res.results[b]["out"] for b in range(B)], axis=0)
    return np.ascontiguousarray(o.reshape(B, H, W, NH * DH).astype(np.float32))



# revision 13
# speedup vs baseline: 1.0424x; 1.0424x over previous
"""AttentionAugmentation2D kernel for 8 Trainium2 NeuronCores — v4.

Data-parallel over batch (B=8 -> 1 batch element per core).

Math (per batch, per head; H=W=32, L=H*W=1024, dh=32):
  logits[(x,y),(x',y')] = q.k + q.krw[y'-y+31] + q.krh[x'-x+31]
Both relative terms are folded into a single K=96 matmul:
  Q_aug = [qT; skew_w(q @ krw^T); skew_h(q @ krh^T)]   (96 x 1024 per head)
  K_aug = [kT; onehot32(y'); onehot32(x')]             (96 x 1024 per head)
logits are computed transposed (keys on partitions) so that exp(logitsT)
is directly the stationary operand of the attention@V matmul.

v4 structure (vs v3):
  - attention@V runs with the weights as the STATIONARY operand and V as
    the moving operand: out[q,d] accumulates over key chunks with only 33
    streamed columns per matmul (ap cost 33 vs 512).  The output lands
    q-on-partitions, which eliminates all 64 PE transposes and the at_sb
    evacuation copies of v3; the softmax denominator rides along as V's
    ones column and normalization is a tiny reciprocal+multiply per head.
  - q/k ship host-pre-transposed and are DMA'd straight into their
    QaugT/KaugT positions (no qkst staging tile, no partition-shift
    copies on DVE/Pool).
  - rel matmuls write two groups per PSUM tile and evacuate with a single
    strided copy per (dir, half, group-pair) on DVE/ACT.
  - exp of the 64 [128,1024] logit tiles is split between ACT (native
    Exp) and DVE ((e^s)^x tensor-tensor pow) to keep both lanes busy.
"""

import math
import numpy as np
import ml_dtypes

import concourse.bass as bass
import concourse.mybir as mybir
import concourse.tile as tile
from concourse import bacc
from concourse.bass_utils import run_bass_kernel_spmd

FP = mybir.dt.float32
BF = mybir.dt.bfloat16
AF = mybir.ActivationFunctionType

B = 8
H = W = 32
NH = 8
DH = 32          # per-head depth for q/k/v
L = H * W        # 1024 positions
SCALE = float(DH) ** -0.5
NT = L // 128    # 8 position tiles


def _build_onehot():
    # rows 0-31: onehot of y' = key % 32 ; rows 32-63: onehot of x' = key//32
    # pre-interleaved to the column layout col = pos*4 + hb, bf16 exact
    oh = np.zeros((64, L), dtype=np.float32)
    k = np.arange(L)
    oh[k % 32, k] = 1.0
    oh[32 + k // 32, k] = 1.0
    ohi = np.repeat(oh[:, :, None], 4, axis=2).reshape(64, 4 * L)
    return np.ascontiguousarray(ohi.astype(ml_dtypes.bfloat16))


def _build_nc():
    nc = bacc.Bacc(
        "TRN2",
        target_bir_lowering=False,
        debug=False,
        enable_asserts=True,
        num_devices=B,
    )
    # q/k depth rows, host-transposed: [32 d, 2 half, L pos, 4 hb]
    qT = nc.declare_dram_parameter("qT", [DH, 2 * L * 4], BF, isOutput=False)
    kT = nc.declare_dram_parameter("kT", [DH, 2 * L * 4], BF, isOutput=False)
    krwh = nc.declare_dram_parameter("krwhT", [DH, 2 * (2 * W - 1)], BF, isOutput=False)
    oneh = nc.declare_dram_parameter("oneh", [64, 4 * L], BF, isOutput=False)
    xv = nc.declare_dram_parameter("xv", [L, NH * DH], BF, isOutput=False)
    out = nc.declare_dram_parameter("out", [L, NH * DH], FP, isOutput=True)

    def copy_on(eng, dst, src):
        if eng is nc.scalar:
            eng.copy(dst, src)
        else:
            eng.tensor_copy(dst, src)

    with tile.TileContext(nc) as tc:
        with (
            tc.tile_pool(name="const", bufs=1) as cp,
        ):
            krwh_sb = cp.tile([DH, 2 * (2 * W - 1)], BF)
            krw_sb = krwh_sb[:, 0 : 2 * W - 1]
            krh_sb = krwh_sb[:, 2 * W - 1 :]

            # interleaved column layout: col(half, pos, hb) =
            #   half*4096 + pos*4 + hb,  head h = half*4 + hb
            QaugT = cp.tile([96, 2, L, 4], BF)
            KaugT = cp.tile([96, 2, L, 4], BF)
            Vaug = cp.tile([128, NT, NH, DH + 2], BF)

            # deadline-ordered DMAs on parallel queues:
            #  ACT queue: krwh (rel matmuls need it first, tiny)
            #  SP queue: q half0 -> k half0 -> oneh half0 -> q/k/oneh half1
            #  Pool (swdge): V
            qTr = qT.rearrange("p (a c) -> p a c", a=2)
            kTr = kT.rearrange("p (a c) -> p a c", a=2)
            with tc.high_priority():
                nc.scalar.dma_start(out=krwh_sb, in_=krwh[:])
                nc.sync.dma_start(
                    out=QaugT[0:32, 0].rearrange("p f h -> p (f h)"), in_=qTr[:, 0]
                )
                nc.sync.dma_start(
                    out=KaugT[0:32, 0].rearrange("p f h -> p (f h)"), in_=kTr[:, 0]
                )
                nc.sync.dma_start(
                    out=KaugT[32:96, 0].rearrange("p f h -> p (f h)"), in_=oneh[:]
                )
            with tc.tile_wait_until(0.004):
                nc.sync.dma_start(
                    out=QaugT[0:32, 1].rearrange("p f h -> p (f h)"), in_=qTr[:, 1]
                )
                nc.sync.dma_start(
                    out=KaugT[0:32, 1].rearrange("p f h -> p (f h)"), in_=kTr[:, 1]
                )
                nc.sync.dma_start(
                    out=KaugT[32:96, 1].rearrange("p f h -> p (f h)"), in_=oneh[:]
                )
            # V straight into its SBUF layout (leaves the ones column gap);
            # per-t pieces keep the DMA APs within 3 dims
            xvr = xv.rearrange("(t p) c -> p t c", p=128)
            with tc.tile_wait_until(0.006):
                for t in range(NT):
                    nc.sync.dma_start(
                        out=Vaug[:, t, :, 0:DH],
                        in_=xvr[:, t, :].rearrange("p (h d) -> p h d", d=DH),
                    )
            # ones column for the softmax denominator: engine memset, no DMA
            nc.vector.memset(
                Vaug[:, :, :, DH : DH + 1].rearrange("p t h o -> p (t h o)"), 1.0
            )

            out_sb = cp.tile([128, NT, NH * DH], FP)
            # (e^SCALE)^logit == exp(SCALE*logit): lets the DVE compute the
            # softmax exp as a TensorTensor pow with a broadcast const base
            ebase = cp.tile([128, 1], FP)
            nc.vector.memset(ebase, math.exp(SCALE))

            # rel views (interleaved): free ordering per mm is (hb, x|y)
            q_i = QaugT[0:32]                                  # [32,2,L,4]
            qr = q_i.rearrange("p a (x y) h -> p a h x y", y=W)
            wd = QaugT[32:64].rearrange("p a (x y) h -> p a h x y", y=W)
            hd = QaugT[64:96].rearrange("p a (x y) h -> p a h x y", y=W)

            def rel_pair(pool, half, gp, wdir, eng, tag="rp"):
                # two y(or x)-groups (8 pre-skewed rel matmuls) for one
                # 4-head half into one PSUM tile, then a single evac
                rp = pool.tile(
                    [32, 2, 4, 4, 32], FP, tag=tag, name=f"rp{half}_{wdir}_{gp}"
                )
                for gg in range(2):
                    g = 2 * gp + gg
                    for i in range(4):
                        v = 4 * g + i
                        if wdir:
                            nc.tensor.matmul(
                                rp[:, gg, i],
                                lhsT=krw_sb[:, 31 - v : 63 - v],
                                rhs=qr[:, half, :, :, v],
                                start=True,
                                stop=True,
                            )
                        else:
                            nc.tensor.matmul(
                                rp[:, gg, i],
                                lhsT=krh_sb[:, 31 - v : 63 - v],
                                rhs=qr[:, half, :, v, :],
                                start=True,
                                stop=True,
                            )
                if wdir:
                    dst = wd[:, half, :, :, 8 * gp : 8 * gp + 8].rearrange(
                        "p h x (gg i) -> p gg i h x", gg=2
                    )
                else:
                    dst = hd[:, half, :, 8 * gp : 8 * gp + 8, :].rearrange(
                        "p h (gg i) y -> p gg i h y", gg=2
                    )
                copy_on(eng, dst, rp)

            # ---------------- rel half 0 (heads 0-3) ----------------------
            with tc.tile_pool(name="ps_rel", bufs=3, space="PSUM") as ps_rel:
                for gp in range(4):
                    rel_pair(ps_rel, 0, gp, True, (nc.vector, nc.scalar)[gp % 2])
                for gp in range(4):
                    rel_pair(ps_rel, 0, gp, False, (nc.scalar, nc.vector)[gp % 2])

            # ---------------- attention over heads ------------------------
            with (
                tc.tile_pool(name="wt", bufs=3) as wtp,
                tc.tile_pool(name="stg", bufs=3) as stp,
                tc.tile_pool(name="ps_lt", bufs=2, space="PSUM") as ps_lt,
                tc.tile_pool(name="ps_av", bufs=2, space="PSUM") as ps_av,
                tc.tile_pool(name="ps_rel1", bufs=1, space="PSUM") as ps_rel1,
            ):
                wts = {}
                avs = {}

                def finish_head(h):
                    av = avs.pop(h)
                    rcp = cp.tile([128, NT], FP, tag="rcp", name=f"rcp{h}")
                    nc.vector.reciprocal(rcp, av[:, :, DH])
                    rcp_b = bass.AP(
                        tensor=rcp.tensor,
                        offset=rcp.offset,
                        ap=[rcp.ap[0], rcp.ap[1], [0, DH]],
                    )
                    nc.vector.tensor_tensor(
                        out_sb[:, :, h * DH : (h + 1) * DH],
                        av[:, :, 0:DH],
                        rcp_b,
                        mybir.AluOpType.mult,
                    )

                # rel half 1 pairs spread over the first heads (heads 4-7
                # only need them from phase 4); evacs on DVE/ACT
                rel1 = [(gp, True) for gp in range(4)] + [(gp, False) for gp in range(4)]

                # exp engine split: ACT is a bit faster per tile than DVE
                # (996 vs 1192 ns); 37/64 on ACT balances the two lanes once
                # the rel evacs and finish multiplies are counted in.
                ACT_TILES = 37
                exp_on_act = [
                    (i * ACT_TILES) // 64 != ((i + 1) * ACT_TILES) // 64
                    for i in range(64)
                ]

                def av_group(h, qt):
                    # one query tile's attention@V: 8 sequential accumulation
                    # matmuls (one pending PSUM group per bank at a time)
                    avp = avs[h]
                    for kt2 in range(NT):
                        nc.tensor.matmul(
                            avp[:, qt, :],
                            lhsT=wts[h][:, kt2, qt * 128 : (qt + 1) * 128],
                            rhs=Vaug[:, kt2, h, 0 : DH + 1],
                            start=(kt2 == 0),
                            stop=(kt2 == NT - 1),
                        )

                for h in range(NH):
                    wts[h] = wtp.tile(
                        [128, NT, L], BF, tag="wt", name=f"wt{h}"
                    )
                    ha, hb = h // 4, h % 4
                    avs[h] = ps_av.tile(
                        [128, NT, DH + 1], FP, tag="av", name=f"av{h}"
                    )
                    for kt in range(NT):
                        if True:
                            lt = ps_lt.tile([128, L], FP, tag="lt")
                            for qc in range(2):
                                nc.tensor.matmul(
                                    lt[:, qc * 512 : (qc + 1) * 512],
                                    lhsT=KaugT[:, ha, kt * 128 : (kt + 1) * 128, hb],
                                    rhs=QaugT[:, ha, qc * 512 : (qc + 1) * 512, hb],
                                    start=True,
                                    stop=True,
                                )
                            if exp_on_act[h * NT + kt]:
                                nc.scalar.activation(
                                    wts[h][:, kt, :], lt, AF.Exp, scale=SCALE
                                )
                            else:
                                # pow is not ISA-legal on DVE: DVE evacuates
                                # the tile, Pool computes (e^SCALE)^x
                                stg = stp.tile([128, L], FP, tag="stg")
                                nc.vector.tensor_copy(stg, lt)
                                eb = bass.AP(
                                    tensor=ebase.tensor,
                                    offset=ebase.offset,
                                    ap=[ebase.ap[0], [0, L]],
                                )
                                nc.gpsimd.tensor_tensor(
                                    wts[h][:, kt, :], eb, stg,
                                    mybir.AluOpType.pow,
                                )
                        # attention@V for the previous head, one query tile
                        # per slot, interleaved with this head's QK
                        if h > 0:
                            av_group(h - 1, kt)
                        if h < 4 and kt in (1, 4):
                            gp, wdir = rel1[2 * h + (kt == 4)]
                            rel_pair(
                                ps_rel1, 1, gp, wdir,
                                (nc.scalar, nc.vector)[(2 * h + (kt == 4)) % 2],
                                tag="r1",
                            )
                    if h > 0:
                        del wts[h - 1]
                        finish_head(h - 1)

                # tail: last head's attention@V and finish
                for qt in range(NT):
                    av_group(NH - 1, qt)
                del wts[NH - 1]
                finish_head(NH - 1)
                # out stores, overlapped two chunks per queue
                out_r = out.rearrange("(t p) c -> p t c", p=128)
                for t in range(0, NT, 2):
                    eng = (nc.sync, nc.gpsimd)[(t // 2) % 2]
                    eng.dma_start(
                        out=out_r[:, t : t + 2, :], in_=out_sb[:, t : t + 2, :]
                    )
    nc.compile()
    return nc


_NC_CACHE = None


def _prep(inputs, key_rel_w, key_rel_h):
    xf32 = inputs.astype(np.float32).reshape(-1, L, 3 * NH * DH)
    nb = xf32.shape[0]
    # [d, half, pos, hb]: channel c = (half*4 + hb)*32 + d
    qk = xf32[:, :, 0:512].transpose(0, 2, 1).reshape(nb, 2, 2, 4, DH, L)
    # qk[b, qk, half, hb, d, pos] -> [b, qk, d, half, pos, hb]
    qk = qk.transpose(0, 1, 4, 2, 5, 3).reshape(nb, 2, DH, 2 * L * 4)
    qT = np.ascontiguousarray(qk[:, 0].astype(ml_dtypes.bfloat16))
    kT = np.ascontiguousarray(qk[:, 1].astype(ml_dtypes.bfloat16))
    xv = np.ascontiguousarray(
        xf32[:, :, 512:768].astype(ml_dtypes.bfloat16)
    )
    krwhT = np.ascontiguousarray(
        np.concatenate([key_rel_w, key_rel_h], axis=0)
        .astype(np.float32).T.astype(ml_dtypes.bfloat16)
    )
    return qT, kT, xv, krwhT


def kernel(inputs: np.ndarray, key_rel_w: np.ndarray, key_rel_h: np.ndarray) -> np.ndarray:
    global _NC_CACHE
    qT, kT, xv, krwhT = _prep(inputs, key_rel_w, key_rel_h)
    oneh = _build_onehot()

    if _NC_CACHE is None:
        _NC_CACHE = _build_nc()
    nc = _NC_CACHE

    in_maps = [
        {"qT": qT[b], "kT": kT[b], "xv": xv[b], "krwhT": krwhT, "oneh": oneh}
        for b in range(B)
    ]
    res = run_bass_kernel_spmd(nc, in_maps, list(range(B)))
    o = np.stack([res.results[b]["out"] for b in range(B)], axis=0)
    return np.ascontiguousarray(o.reshape(B, H, W, NH * DH).astype(np.float32))


# revision 21
# speedup vs baseline: 1.2452x; 1.1945x over previous
"""AttentionAugmentation2D kernel for 8 Trainium2 NeuronCores — v4.

Data-parallel over batch (B=8 -> 1 batch element per core).

Math (per batch, per head; H=W=32, L=H*W=1024, dh=32):
  logits[(x,y),(x',y')] = q.k + q.krw[y'-y+31] + q.krh[x'-x+31]
Both relative terms are folded into a single K=96 matmul:
  Q_aug = [qT; skew_w(q @ krw^T); skew_h(q @ krh^T)]   (96 x 1024 per head)
  K_aug = [kT; onehot32(y'); onehot32(x')]             (96 x 1024 per head)
logits are computed transposed (keys on partitions) so that exp(logitsT)
is directly the stationary operand of the attention@V matmul.

v4 structure (vs v3):
  - attention@V runs with the weights as the STATIONARY operand and V as
    the moving operand: out[q,d] accumulates over key chunks with only 33
    streamed columns per matmul (ap cost 33 vs 512).  The output lands
    q-on-partitions, which eliminates all 64 PE transposes and the at_sb
    evacuation copies of v3; the softmax denominator rides along as V's
    ones column and normalization is a tiny reciprocal+multiply per head.
  - q/k ship host-pre-transposed and are DMA'd straight into their
    QaugT/KaugT positions (no qkst staging tile, no partition-shift
    copies on DVE/Pool).
  - rel matmuls write two groups per PSUM tile and evacuate with a single
    strided copy per (dir, half, group-pair) on DVE/ACT.
  - exp of the 64 [128,1024] logit tiles is split between ACT (native
    Exp) and DVE ((e^s)^x tensor-tensor pow) to keep both lanes busy.
"""

import math
import numpy as np
import ml_dtypes

import concourse.bass as bass
import concourse.mybir as mybir
import concourse.tile as tile
from concourse import bacc
from concourse.bass_utils import run_bass_kernel_spmd

FP = mybir.dt.float32
BF = mybir.dt.bfloat16
AF = mybir.ActivationFunctionType

B = 8
H = W = 32
NH = 8
DH = 32          # per-head depth for q/k/v
L = H * W        # 1024 positions
SCALE = float(DH) ** -0.5
NT = L // 128    # 8 position tiles


def _build_onehot():
    # rows 0-31: onehot of y' = key % 32 ; rows 32-63: onehot of x' = key//32
    # pre-interleaved to the column layout col = pos*4 + hb, bf16 exact
    oh = np.zeros((64, L), dtype=np.float32)
    k = np.arange(L)
    oh[k % 32, k] = 1.0
    oh[32 + k // 32, k] = 1.0
    ohi = np.repeat(oh[:, :, None], 4, axis=2).reshape(64, 4 * L)
    return np.ascontiguousarray(ohi.astype(ml_dtypes.bfloat16))


def _build_nc():
    nc = bacc.Bacc(
        "TRN2",
        target_bir_lowering=False,
        debug=False,
        enable_asserts=True,
        num_devices=B,
    )
    # q/k depth rows host-transposed+interleaved, packed as four 32-row
    # groups (qh0, qh1, kh0, kh1) across 128 partitions: DMA cost is
    # per-partition bytes, so one 128-wide blob beats four 32-wide DMAs 4x
    qki = nc.declare_dram_parameter("qki", [128, 4 * L], BF, isOutput=False)
    krwh = nc.declare_dram_parameter("krwhT", [DH, 2 * (2 * W - 1)], BF, isOutput=False)
    oneh = nc.declare_dram_parameter("oneh", [64, 4 * L], BF, isOutput=False)
    xv = nc.declare_dram_parameter("xv", [L, NH * DH], BF, isOutput=False)
    out = nc.declare_dram_parameter("out", [L, NH * DH], FP, isOutput=True)

    def copy_on(eng, dst, src):
        if eng is nc.scalar:
            eng.copy(dst, src)
        else:
            eng.tensor_copy(dst, src)

    with tile.TileContext(nc) as tc:
        with (
            tc.tile_pool(name="const", bufs=1) as cp,
        ):
            krwh_sb = cp.tile([DH, 2 * (2 * W - 1)], BF)
            krw_sb = krwh_sb[:, 0 : 2 * W - 1]
            krh_sb = krwh_sb[:, 2 * W - 1 :]

            # interleaved column layout: col(half, pos, hb) =
            #   half*4096 + pos*4 + hb,  head h = half*4 + hb
            QaugT = cp.tile([96, 2, L, 4], BF)
            KaugT = cp.tile([96, 2, L, 4], BF)
            Vaug = cp.tile([128, NT, NH, DH + 2], BF)

            # deadline-ordered DMAs on parallel queues:
            #  ACT queue: krwh (rel matmuls need it first, tiny)
            #  SP queue: q half0 -> k half0 -> oneh half0 -> q/k/oneh half1
            #  Pool (swdge): V
            qkst = cp.tile([128, 4 * L], BF, name="qkst")
            with tc.high_priority():
                nc.scalar.dma_start(out=krwh_sb, in_=krwh[:])
                nc.sync.dma_start(out=qkst, in_=qki[:])
                nc.sync.dma_start(
                    out=KaugT[32:96, 0].rearrange("p f h -> p (f h)"), in_=oneh[:]
                )
            with tc.tile_wait_until(0.004):
                nc.sync.dma_start(
                    out=KaugT[32:96, 1].rearrange("p f h -> p (f h)"), in_=oneh[:]
                )
            # V straight into its SBUF layout (leaves the ones column gap);
            # per-t pieces keep the DMA APs within 3 dims
            xvr = xv.rearrange("(t p) c -> p t c", p=128)
            with tc.tile_wait_until(0.006):
                for t in range(NT):
                    nc.sync.dma_start(
                        out=Vaug[:, t, :, 0:DH],
                        in_=xvr[:, t, :].rearrange("p (h d) -> p h d", d=DH),
                    )
            # ones column for the softmax denominator: engine memset, no DMA
            nc.vector.memset(
                Vaug[:, :, :, DH : DH + 1].rearrange("p t h o -> p (t h o)"), 1.0
            )

            # partition-shift redistribution of the qki blob: half-0 rows on
            # DVE (fast, needed first), half-1 on Pool (idle early)
            nc.vector.tensor_copy(
                QaugT[0:32, 0].rearrange("p f h -> p (f h)"), qkst[0:32, :]
            )
            nc.vector.tensor_copy(
                KaugT[0:32, 0].rearrange("p f h -> p (f h)"), qkst[64:96, :]
            )
            nc.gpsimd.tensor_copy(
                QaugT[0:32, 1].rearrange("p f h -> p (f h)"), qkst[32:64, :]
            )
            nc.gpsimd.tensor_copy(
                KaugT[0:32, 1].rearrange("p f h -> p (f h)"), qkst[96:128, :]
            )

            out_sb = cp.tile([128, NT, NH * DH], FP)
            # (e^SCALE)^logit == exp(SCALE*logit): lets the DVE compute the
            # softmax exp as a TensorTensor pow with a broadcast const base
            ebase = cp.tile([128, 1], FP)
            nc.vector.memset(ebase, math.exp(SCALE))

            # rel views (interleaved): free ordering per mm is (hb, x|y)
            q_i = QaugT[0:32]                                  # [32,2,L,4]
            qr = q_i.rearrange("p a (x y) h -> p a h x y", y=W)
            wd = QaugT[32:64].rearrange("p a (x y) h -> p a h x y", y=W)
            hd = QaugT[64:96].rearrange("p a (x y) h -> p a h x y", y=W)

            def rel_pair(pool, half, gp, wdir, eng, tag="rp"):
                # two y(or x)-groups (8 pre-skewed rel matmuls) for one
                # 4-head half into one PSUM tile, then a single evac
                rp = pool.tile(
                    [32, 2, 4, 4, 32], FP, tag=tag, name=f"rp{half}_{wdir}_{gp}"
                )
                for gg in range(2):
                    g = 2 * gp + gg
                    for i in range(4):
                        v = 4 * g + i
                        if wdir:
                            nc.tensor.matmul(
                                rp[:, gg, i],
                                lhsT=krw_sb[:, 31 - v : 63 - v],
                                rhs=qr[:, half, :, :, v],
                                start=True,
                                stop=True,
                            )
                        else:
                            nc.tensor.matmul(
                                rp[:, gg, i],
                                lhsT=krh_sb[:, 31 - v : 63 - v],
                                rhs=qr[:, half, :, v, :],
                                start=True,
                                stop=True,
                            )
                if wdir:
                    dst = wd[:, half, :, :, 8 * gp : 8 * gp + 8].rearrange(
                        "p h x (gg i) -> p gg i h x", gg=2
                    )
                else:
                    dst = hd[:, half, :, 8 * gp : 8 * gp + 8, :].rearrange(
                        "p h (gg i) y -> p gg i h y", gg=2
                    )
                copy_on(eng, dst, rp)

            # ---------------- rel half 0 (heads 0-3) ----------------------
            with tc.tile_pool(name="ps_rel", bufs=3, space="PSUM") as ps_rel:
                for gp in range(4):
                    rel_pair(ps_rel, 0, gp, True, (nc.vector, nc.scalar)[gp % 2])
                for gp in range(4):
                    rel_pair(ps_rel, 0, gp, False, (nc.scalar, nc.vector)[gp % 2])

            # ---------------- attention over heads ------------------------
            with (
                tc.tile_pool(name="wt", bufs=3) as wtp,
                tc.tile_pool(name="stg", bufs=3) as stp,
                tc.tile_pool(name="ps_lt", bufs=3, space="PSUM") as ps_lt,
                tc.tile_pool(name="ps_av", bufs=2, space="PSUM") as ps_av,
            ):
                wts = {}
                avs = {}

                def finish_head(h):
                    av = avs.pop(h)
                    rcp = cp.tile([128, NT], FP, tag="rcp", name=f"rcp{h}")
                    nc.vector.reciprocal(rcp, av[:, :, DH])
                    rcp_b = bass.AP(
                        tensor=rcp.tensor,
                        offset=rcp.offset,
                        ap=[rcp.ap[0], rcp.ap[1], [0, DH]],
                    )
                    nc.vector.tensor_tensor(
                        out_sb[:, :, h * DH : (h + 1) * DH],
                        av[:, :, 0:DH],
                        rcp_b,
                        mybir.AluOpType.mult,
                    )

                # rel half 1 pairs spread over the first heads (heads 4-7
                # only need them from phase 4); evacs on DVE/ACT
                rel1 = [(gp, True) for gp in range(4)] + [(gp, False) for gp in range(4)]

                # exp engine split: ACT is a bit faster per tile than DVE
                # (996 vs 1192 ns); 37/64 on ACT balances the two lanes once
                # the rel evacs and finish multiplies are counted in.
                ACT_TILES = 36
                exp_on_act = [
                    (i * ACT_TILES) // 64 != ((i + 1) * ACT_TILES) // 64
                    for i in range(64)
                ]

                def av_group(h, qt):
                    # one query tile's attention@V: 8 sequential accumulation
                    # matmuls (one pending PSUM group per bank at a time)
                    avp = avs[h]
                    for kt2 in range(NT):
                        nc.tensor.matmul(
                            avp[:, qt, 0 : DH + 1],
                            lhsT=wts[h][:, kt2, qt * 128 : (qt + 1) * 128],
                            rhs=Vaug[:, kt2, h, 0 : DH + 1],
                            start=(kt2 == 0),
                            stop=(kt2 == NT - 1),
                        )

                for h in range(NH):
                    wts[h] = wtp.tile(
                        [128, NT, L], BF, tag="wt", name=f"wt{h}"
                    )
                    ha, hb = h // 4, h % 4
                    # padded to one full 2KB PSUM bank so two heads' pending
                    # accumulation groups never share a zero region
                    avs[h] = ps_av.tile(
                        [128, NT, 64], FP, tag="av", name=f"av{h}"
                    )
                    for kt in range(NT):
                        if True:
                            lt = ps_lt.tile([128, L], FP, tag="lt")
                            for qc in range(2):
                                nc.tensor.matmul(
                                    lt[:, qc * 512 : (qc + 1) * 512],
                                    lhsT=KaugT[:, ha, kt * 128 : (kt + 1) * 128, hb],
                                    rhs=QaugT[:, ha, qc * 512 : (qc + 1) * 512, hb],
                                    start=True,
                                    stop=True,
                                )
                            if exp_on_act[h * NT + kt]:
                                nc.scalar.activation(
                                    wts[h][:, kt, :], lt, AF.Exp, scale=SCALE
                                )
                            else:
                                # pow is not ISA-legal on DVE: DVE evacuates
                                # the tile, Pool computes (e^SCALE)^x
                                stg = stp.tile([128, L], FP, tag="stg")
                                nc.vector.tensor_copy(stg, lt)
                                eb = bass.AP(
                                    tensor=ebase.tensor,
                                    offset=ebase.offset,
                                    ap=[ebase.ap[0], [0, L]],
                                )
                                nc.gpsimd.tensor_tensor(
                                    wts[h][:, kt, :], eb, stg,
                                    mybir.AluOpType.pow,
                                )
                        # attention@V for the previous head, one query tile
                        # per slot, interleaved with this head's QK
                        if h > 0:
                            av_group(h - 1, kt)
                        # rel half 1 during heads 0-1, riding the lt ring
                        # (same per-partition PSUM size)
                        if h < 2 and kt % 2 == 1:
                            slot = 4 * h + kt // 2
                            gp, wdir = rel1[slot]
                            rel_pair(
                                ps_lt, 1, gp, wdir,
                                (nc.scalar, nc.vector)[slot % 2],
                                tag="lt",
                            )
                    if h > 0:
                        del wts[h - 1]
                        finish_head(h - 1)

                # tail: last head's attention@V and finish
                for qt in range(NT):
                    av_group(NH - 1, qt)
                del wts[NH - 1]
                finish_head(NH - 1)
                # out stores, overlapped two chunks per queue
                out_r = out.rearrange("(t p) c -> p t c", p=128)
                for t in range(0, NT, 2):
                    eng = (nc.sync, nc.gpsimd)[(t // 2) % 2]
                    eng.dma_start(
                        out=out_r[:, t : t + 2, :], in_=out_sb[:, t : t + 2, :]
                    )
    nc.compile()
    return nc


_NC_CACHE = None


def _prep(inputs, key_rel_w, key_rel_h):
    xf32 = inputs.astype(np.float32).reshape(-1, L, 3 * NH * DH)
    nb = xf32.shape[0]
    # [g, hb, d, pos] -> [g*32+d, pos*4 + hb], groups (qh0, qh1, kh0, kh1)
    qki = np.ascontiguousarray(
        xf32[:, :, 0:512].transpose(0, 2, 1).reshape(nb, 4, 4, DH, L)
        .transpose(0, 1, 3, 4, 2).reshape(nb, 128, 4 * L)
        .astype(ml_dtypes.bfloat16)
    )
    xv = np.ascontiguousarray(
        xf32[:, :, 512:768].astype(ml_dtypes.bfloat16)
    )
    krwhT = np.ascontiguousarray(
        np.concatenate([key_rel_w, key_rel_h], axis=0)
        .astype(np.float32).T.astype(ml_dtypes.bfloat16)
    )
    return qki, xv, krwhT


def kernel(inputs: np.ndarray, key_rel_w: np.ndarray, key_rel_h: np.ndarray) -> np.ndarray:
    global _NC_CACHE
    qki, xv, krwhT = _prep(inputs, key_rel_w, key_rel_h)
    oneh = _build_onehot()

    if _NC_CACHE is None:
        _NC_CACHE = _build_nc()
    nc = _NC_CACHE

    in_maps = [
        {"qki": qki[b], "xv": xv[b], "krwhT": krwhT, "oneh": oneh}
        for b in range(B)
    ]
    res = run_bass_kernel_spmd(nc, in_maps, list(range(B)))
    o = np.stack([res.results[b]["out"] for b in range(B)], axis=0)
    return np.ascontiguousarray(o.reshape(B, H, W, NH * DH).astype(np.float32))


# revision 28
# speedup vs baseline: 1.2759x; 1.0247x over previous
"""AttentionAugmentation2D kernel for 8 Trainium2 NeuronCores — v4.

Data-parallel over batch (B=8 -> 1 batch element per core).

Math (per batch, per head; H=W=32, L=H*W=1024, dh=32):
  logits[(x,y),(x',y')] = q.k + q.krw[y'-y+31] + q.krh[x'-x+31]
Both relative terms are folded into a single K=96 matmul:
  Q_aug = [qT; skew_w(q @ krw^T); skew_h(q @ krh^T)]   (96 x 1024 per head)
  K_aug = [kT; onehot32(y'); onehot32(x')]             (96 x 1024 per head)
logits are computed transposed (keys on partitions) so that exp(logitsT)
is directly the stationary operand of the attention@V matmul.

v4 structure (vs v3):
  - attention@V runs with the weights as the STATIONARY operand and V as
    the moving operand: out[q,d] accumulates over key chunks with only 33
    streamed columns per matmul (ap cost 33 vs 512).  The output lands
    q-on-partitions, which eliminates all 64 PE transposes and the at_sb
    evacuation copies of v3; the softmax denominator rides along as V's
    ones column and normalization is a tiny reciprocal+multiply per head.
  - q/k ship host-pre-transposed and are DMA'd straight into their
    QaugT/KaugT positions (no qkst staging tile, no partition-shift
    copies on DVE/Pool).
  - rel matmuls write two groups per PSUM tile and evacuate with a single
    strided copy per (dir, half, group-pair) on DVE/ACT.
  - exp of the 64 [128,1024] logit tiles is split between ACT (native
    Exp) and DVE ((e^s)^x tensor-tensor pow) to keep both lanes busy.
"""

import math
import numpy as np
import ml_dtypes

import concourse.bass as bass
import concourse.mybir as mybir
import concourse.tile as tile
from concourse import bacc
from concourse.bass_utils import run_bass_kernel_spmd

FP = mybir.dt.float32
BF = mybir.dt.bfloat16
AF = mybir.ActivationFunctionType

B = 8
H = W = 32
NH = 8
DH = 32          # per-head depth for q/k/v
L = H * W        # 1024 positions
SCALE = float(DH) ** -0.5
NT = L // 128    # 8 position tiles


def _build_onehot():
    # rows 0-31: onehot of y' = key % 32 ; rows 32-63: onehot of x' = key//32
    # pre-interleaved to the column layout col = pos*4 + hb, bf16 exact
    oh = np.zeros((64, L), dtype=np.float32)
    k = np.arange(L)
    oh[k % 32, k] = 1.0
    oh[32 + k // 32, k] = 1.0
    ohi = np.repeat(oh[:, :, None], 4, axis=2).reshape(64, 4 * L)
    return np.ascontiguousarray(ohi.astype(ml_dtypes.bfloat16))


def _build_nc():
    nc = bacc.Bacc(
        "TRN2",
        target_bir_lowering=False,
        debug=False,
        enable_asserts=True,
        num_devices=B,
    )
    # q/k depth rows host-transposed+interleaved, packed as four 32-row
    # groups (qh0, qh1, kh0, kh1) across 128 partitions: DMA cost is
    # per-partition bytes, so one 128-wide blob beats four 32-wide DMAs 4x
    qki = nc.declare_dram_parameter("qki", [128, 4 * L], BF, isOutput=False)
    krwh = nc.declare_dram_parameter("krwhT", [DH, 2 * (2 * W - 1)], BF, isOutput=False)
    oneh = nc.declare_dram_parameter("oneh", [64, 4 * L], BF, isOutput=False)
    xv = nc.declare_dram_parameter("xv", [L, NH * DH], BF, isOutput=False)
    out = nc.declare_dram_parameter("out", [L, NH * DH], FP, isOutput=True)

    def copy_on(eng, dst, src):
        if eng is nc.scalar:
            eng.copy(dst, src)
        else:
            eng.tensor_copy(dst, src)

    with tile.TileContext(nc) as tc:
        with (
            tc.tile_pool(name="const", bufs=1) as cp,
        ):
            krwh_sb = cp.tile([DH, 2 * (2 * W - 1)], BF)
            krw_sb = krwh_sb[:, 0 : 2 * W - 1]
            krh_sb = krwh_sb[:, 2 * W - 1 :]

            # interleaved column layout: col(half, pos, hb) =
            #   half*4096 + pos*4 + hb,  head h = half*4 + hb
            QaugT = cp.tile([96, 2, L, 4], BF)
            KaugT = cp.tile([96, 2, L, 4], BF)
            Vaug = cp.tile([128, NT, NH, DH + 2], BF)

            # deadline-ordered DMAs on parallel queues:
            #  ACT queue: krwh (rel matmuls need it first, tiny)
            #  SP queue: q half0 -> k half0 -> oneh half0 -> q/k/oneh half1
            #  Pool (swdge): V
            qkst = cp.tile([128, 4 * L], BF, name="qkst")
            with tc.high_priority():
                nc.scalar.dma_start(out=krwh_sb, in_=krwh[:])
                nc.sync.dma_start(out=qkst, in_=qki[:])
                nc.sync.dma_start(
                    out=KaugT[32:96, 0].rearrange("p f h -> p (f h)"), in_=oneh[:]
                )
            with tc.tile_wait_until(0.004):
                nc.sync.dma_start(
                    out=KaugT[32:96, 1].rearrange("p f h -> p (f h)"), in_=oneh[:]
                )
            # V straight into its SBUF layout (leaves the ones column gap);
            # per-t pieces keep the DMA APs within 3 dims
            xvr = xv.rearrange("(t p) c -> p t c", p=128)
            with tc.tile_wait_until(0.006):
                for t in range(NT):
                    nc.sync.dma_start(
                        out=Vaug[:, t, :, 0:DH],
                        in_=xvr[:, t, :].rearrange("p (h d) -> p h d", d=DH),
                    )
            # ones column for the softmax denominator: engine memset, no DMA
            nc.vector.memset(
                Vaug[:, :, :, DH : DH + 1].rearrange("p t h o -> p (t h o)"), 1.0
            )

            # partition-shift redistribution of the qki blob: half-0 rows on
            # DVE (fast, needed first), half-1 on Pool (idle early)
            nc.vector.tensor_copy(
                QaugT[0:32, 0].rearrange("p f h -> p (f h)"), qkst[0:32, :]
            )
            nc.vector.tensor_copy(
                KaugT[0:32, 0].rearrange("p f h -> p (f h)"), qkst[64:96, :]
            )
            nc.gpsimd.tensor_copy(
                QaugT[0:32, 1].rearrange("p f h -> p (f h)"), qkst[32:64, :]
            )
            nc.gpsimd.tensor_copy(
                KaugT[0:32, 1].rearrange("p f h -> p (f h)"), qkst[96:128, :]
            )

            out_sb = cp.tile([128, NT, NH * DH], FP)
            # (e^SCALE)^logit == exp(SCALE*logit): lets the DVE compute the
            # softmax exp as a TensorTensor pow with a broadcast const base
            ebase = cp.tile([128, 1], FP)
            nc.vector.memset(ebase, math.exp(SCALE))
            # dummy exp pulls the ACT function-table load into the DMA wait
            junk = cp.tile([128, 1], FP)
            with tc.high_priority():
                nc.scalar.activation(junk, ebase, AF.Exp, scale=SCALE)

            # rel views (interleaved): free ordering per mm is (hb, x|y)
            q_i = QaugT[0:32]                                  # [32,2,L,4]
            qr = q_i.rearrange("p a (x y) h -> p a h x y", y=W)
            wd = QaugT[32:64].rearrange("p a (x y) h -> p a h x y", y=W)
            hd = QaugT[64:96].rearrange("p a (x y) h -> p a h x y", y=W)

            # rel groups are kt-aligned so the aug rows stream just in time:
            # QK for key chunk kt covers x' in [4kt,4kt+4), which needs the
            # w-rows for x-chunk kt//2 and the h-rows for v-group kt only.
            def rel_w_chunk(pool, half, xc, eng, tag="rp"):
                # all 32 y-windows restricted to 8 x columns, one PSUM tile
                rp = pool.tile(
                    [32, 32, 4, 8], FP, tag=tag, name=f"rw{half}_{xc}"
                )
                for v in range(W):
                    nc.tensor.matmul(
                        rp[:, v],
                        lhsT=krw_sb[:, 31 - v : 63 - v],
                        rhs=qr[:, half, :, 8 * xc : 8 * xc + 8, v],
                        start=True,
                        stop=True,
                    )
                dst = wd[:, half, :, 8 * xc : 8 * xc + 8, :].rearrange(
                    "p h x y -> p y h x"
                )
                copy_on(eng, dst, rp)

            def rel_h_group(pool, half, g, eng, tag="rp"):
                # one x-window group of 4 pre-skewed rel matmuls
                rp = pool.tile([32, 4, 4, 32], FP, tag=tag, name=f"rh{half}_{g}")
                for i in range(4):
                    v = 4 * g + i
                    nc.tensor.matmul(
                        rp[:, i],
                        lhsT=krh_sb[:, 31 - v : 63 - v],
                        rhs=qr[:, half, :, v, :],
                        start=True,
                        stop=True,
                    )
                dst = hd[:, half, :, 4 * g : 4 * g + 4, :].rearrange(
                    "p h i y -> p i h y"
                )
                copy_on(eng, dst, rp)

            def rel_h_pair(pool, half, gp, eng, tag="rp"):
                # two x-window groups in one PSUM tile (for half 1, where
                # streaming granularity doesn't matter), single evac
                rp = pool.tile(
                    [32, 2, 4, 4, 32], FP, tag=tag, name=f"rhp{half}_{gp}"
                )
                for gg in range(2):
                    for i in range(4):
                        v = 4 * (2 * gp + gg) + i
                        nc.tensor.matmul(
                            rp[:, gg, i],
                            lhsT=krh_sb[:, 31 - v : 63 - v],
                            rhs=qr[:, half, :, v, :],
                            start=True,
                            stop=True,
                        )
                dst = hd[:, half, :, 8 * gp : 8 * gp + 8, :].rearrange(
                    "p h (gg i) y -> p gg i h y", gg=2
                )
                copy_on(eng, dst, rp)

            # ---------------- rel half 0 (heads 0-3) ----------------------
            # issue order unblocks QK kt 0,1 first
            with tc.tile_pool(name="ps_rel", bufs=2, space="PSUM") as ps_rel:
                for xc in range(4):
                    rel_w_chunk(ps_rel, 0, xc, (nc.vector, nc.scalar)[xc % 2], tag="rw")
                    rel_h_group(ps_rel, 0, 2 * xc, nc.scalar, tag="rh")
                    rel_h_group(ps_rel, 0, 2 * xc + 1, nc.vector, tag="rh")

            # ---------------- attention over heads ------------------------
            with (
                tc.tile_pool(name="wt", bufs=3) as wtp,
                tc.tile_pool(name="stg", bufs=3) as stp,
                tc.tile_pool(name="ps_lt", bufs=3, space="PSUM") as ps_lt,
                tc.tile_pool(name="ps_av", bufs=2, space="PSUM") as ps_av,
            ):
                wts = {}
                avs = {}

                out_r = out.rearrange("(t p) c -> p t c", p=128)

                def finish_head(h):
                    av = avs.pop(h)
                    rcp = cp.tile([128, NT], FP, tag="rcp", name=f"rcp{h}")
                    nc.vector.reciprocal(rcp, av[:, :, DH])
                    rcp_b = bass.AP(
                        tensor=rcp.tensor,
                        offset=rcp.offset,
                        ap=[rcp.ap[0], rcp.ap[1], [0, DH]],
                    )
                    nc.vector.tensor_tensor(
                        out_sb[:, :, h * DH : (h + 1) * DH],
                        av[:, :, 0:DH],
                        rcp_b,
                        mybir.AluOpType.mult,
                    )
                    # stream this head's output columns out immediately
                    eng = (nc.sync, nc.gpsimd)[h % 2]
                    eng.dma_start(
                        out=out_r[:, :, h * DH : (h + 1) * DH],
                        in_=out_sb[:, :, h * DH : (h + 1) * DH],
                    )

                # rel half 1 pieces spread over heads 0-1 (heads 4-7 only
                # need them later); they ride the lt PSUM ring (same size)
                rel1 = [(xc, True) for xc in range(4)] + [(gp, False) for gp in range(4)]

                # exp engine split: ACT is a bit faster per tile than DVE
                # (996 vs 1192 ns); 37/64 on ACT balances the two lanes once
                # the rel evacs and finish multiplies are counted in.
                ACT_TILES = 36
                exp_on_act = [
                    (i * ACT_TILES) // 64 != ((i + 1) * ACT_TILES) // 64
                    for i in range(64)
                ]

                def av_group(h, qt):
                    # one query tile's attention@V: 8 sequential accumulation
                    # matmuls (one pending PSUM group per bank at a time)
                    avp = avs[h]
                    for kt2 in range(NT):
                        nc.tensor.matmul(
                            avp[:, qt, 0 : DH + 1],
                            lhsT=wts[h][:, kt2, qt * 128 : (qt + 1) * 128],
                            rhs=Vaug[:, kt2, h, 0 : DH + 1],
                            start=(kt2 == 0),
                            stop=(kt2 == NT - 1),
                        )

                for h in range(NH):
                    wts[h] = wtp.tile(
                        [128, NT, L], BF, tag="wt", name=f"wt{h}"
                    )
                    ha, hb = h // 4, h % 4
                    # padded to one full 2KB PSUM bank so two heads' pending
                    # accumulation groups never share a zero region
                    avs[h] = ps_av.tile(
                        [128, NT, 64], FP, tag="av", name=f"av{h}"
                    )
                    for kt in range(NT):
                        if True:
                            lt = ps_lt.tile([128, L], FP, tag="lt")
                            for qc in range(2):
                                nc.tensor.matmul(
                                    lt[:, qc * 512 : (qc + 1) * 512],
                                    lhsT=KaugT[:, ha, kt * 128 : (kt + 1) * 128, hb],
                                    rhs=QaugT[:, ha, qc * 512 : (qc + 1) * 512, hb],
                                    start=True,
                                    stop=True,
                                )
                            if exp_on_act[h * NT + kt]:
                                nc.scalar.activation(
                                    wts[h][:, kt, :], lt, AF.Exp, scale=SCALE
                                )
                            else:
                                # pow is not ISA-legal on DVE: DVE evacuates
                                # the tile, Pool computes (e^SCALE)^x
                                stg = stp.tile([128, L], FP, tag="stg")
                                nc.vector.tensor_copy(stg, lt)
                                eb = bass.AP(
                                    tensor=ebase.tensor,
                                    offset=ebase.offset,
                                    ap=[ebase.ap[0], [0, L]],
                                )
                                nc.gpsimd.tensor_tensor(
                                    wts[h][:, kt, :], eb, stg,
                                    mybir.AluOpType.pow,
                                )
                        # attention@V for the previous head, one query tile
                        # per slot, interleaved with this head's QK
                        if h > 0:
                            av_group(h - 1, kt)
                        # rel half 1 during heads 0-1, riding the lt ring
                        # (same per-partition PSUM size)
                        if h < 2 and kt % 2 == 1:
                            slot = 4 * h + kt // 2
                            g, wdir = rel1[slot]
                            eng = (nc.scalar, nc.vector)[slot % 2]
                            if wdir:
                                rel_w_chunk(ps_lt, 1, g, eng, tag="lt")
                            else:
                                rel_h_pair(ps_lt, 1, g, eng, tag="lt")
                    if h > 0:
                        del wts[h - 1]
                        finish_head(h - 1)

                # tail: last head's attention@V and finish
                for qt in range(NT):
                    av_group(NH - 1, qt)
                del wts[NH - 1]
                finish_head(NH - 1)
    nc.compile()
    return nc


_NC_CACHE = None


def _prep(inputs, key_rel_w, key_rel_h):
    xf32 = inputs.astype(np.float32).reshape(-1, L, 3 * NH * DH)
    nb = xf32.shape[0]
    # [g, hb, d, pos] -> [g*32+d, pos*4 + hb], groups (qh0, qh1, kh0, kh1)
    qki = np.ascontiguousarray(
        xf32[:, :, 0:512].transpose(0, 2, 1).reshape(nb, 4, 4, DH, L)
        .transpose(0, 1, 3, 4, 2).reshape(nb, 128, 4 * L)
        .astype(ml_dtypes.bfloat16)
    )
    xv = np.ascontiguousarray(
        xf32[:, :, 512:768].astype(ml_dtypes.bfloat16)
    )
    krwhT = np.ascontiguousarray(
        np.concatenate([key_rel_w, key_rel_h], axis=0)
        .astype(np.float32).T.astype(ml_dtypes.bfloat16)
    )
    return qki, xv, krwhT


def kernel(inputs: np.ndarray, key_rel_w: np.ndarray, key_rel_h: np.ndarray) -> np.ndarray:
    global _NC_CACHE
    qki, xv, krwhT = _prep(inputs, key_rel_w, key_rel_h)
    oneh = _build_onehot()

    if _NC_CACHE is None:
        _NC_CACHE = _build_nc()
    nc = _NC_CACHE

    in_maps = [
        {"qki": qki[b], "xv": xv[b], "krwhT": krwhT, "oneh": oneh}
        for b in range(B)
    ]
    res = run_bass_kernel_spmd(nc, in_maps, list(range(B)))
    o = np.stack([res.results[b]["out"] for b in range(B)], axis=0)
    return np.ascontiguousarray(o.reshape(B, H, W, NH * DH).astype(np.float32))
